# revision 3
# baseline (speedup 1.0000x reference)
"""Kernel-correlation (gnn_message_passing) Trainium2 kernel.

out[i, m] = (1/128) * sum_{l<16} exp(-||normal[i] - kernel[m, l]||^2)

Strategy (data-parallel over points, 8 NeuronCores, no collectives):
  -d2[i, j] = 2 x_i . k_j - |x_i|^2 - |k_j|^2 is a rank-13 product of two
  bf16 hi/lo-split augmented matrices (fp32-grade accuracy at full bf16 PE
  rate), computed straight into PSUM by the TensorEngine.  ScalarE then does
  exp(psum - ln 128) PSUM->SBUF in one pass (the 1/128 output scale rides the
  free activation bias), and VectorE does the grouped 16->1 reduction via a
  strided tensor_reduce.  Output rows DMA back contiguously.

Per core: 32768 points x 1024 kernel-points = 33.5M exp evals; ACT floor
~(N+352)/1.2GHz, DVE reduce floor ~N/0.96GHz -> ~280us/core expected.
"""

import math

import numpy as np

N_TOTAL = 262144
N_CORES = 8
N_LOCAL = N_TOTAL // N_CORES  # 32768
M_KERN = 64
K_SUB = 16
MK = M_KERN * K_SUB  # 1024
N_ROWS = 15  # 9 hi/lo cross terms + n2 hi/lo + k2 hi/lo + ln128 hi/lo
CHUNK_PTS = 2048  # points per input DMA chunk
ITER_PTS = 256  # points per PSUM iteration (2 tiles of 128)

TRACE = False  # set by test.py to collect a neuron profile
LAST_RESULTS = None  # BassKernelResults of the most recent run

_CACHED_NC = None


def _build_bass():
    import concourse.bacc as bacc
    import concourse.mybir as mybir
    from concourse.tile import TileContext

    f32 = mybir.dt.float32
    bf16 = mybir.dt.bfloat16
    EXP = mybir.ActivationFunctionType.Exp

    nc = bacc.Bacc()
    xa = nc.declare_dram_parameter("xa", [N_ROWS, N_LOCAL], bf16, isOutput=False)
    ka = nc.declare_dram_parameter("ka", [N_ROWS, MK], bf16, isOutput=False)
    out = nc.declare_dram_parameter("out", [N_LOCAL, M_KERN], f32, isOutput=True)

    with TileContext(nc) as tc:
        with (
            tc.tile_pool(name="kap", bufs=1) as kap,
            tc.tile_pool(name="xap", bufs=2) as xap,
            tc.tile_pool(name="valsp", bufs=2) as valsp,
            tc.tile_pool(name="tre1p", bufs=2) as tre1p,
            tc.tile_pool(name="tre2p", bufs=2) as tre2p,
            tc.tile_pool(name="tre3p", bufs=2) as tre3p,
            tc.tile_pool(name="outp", bufs=3) as outp,
            tc.tile_pool(name="psump", bufs=2, space="PSUM") as psump,
        ):
            kat = kap.tile([N_ROWS, MK], bf16)
            nc.gpsimd.dma_start(out=kat[:], in_=ka[:])

            for c in range(N_LOCAL // CHUNK_PTS):
                xat = xap.tile([N_ROWS, CHUNK_PTS], bf16)
                nc.gpsimd.dma_start(
                    out=xat[:], in_=xa[:, c * CHUNK_PTS : (c + 1) * CHUNK_PTS]
                )
                for it in range(CHUNK_PTS // ITER_PTS):
                    p0 = it * ITER_PTS  # offset within the chunk
                    g0 = c * CHUNK_PTS + p0  # global row offset for this core
                    ps = psump.tile([128, 2 * MK], f32)
                    for half in range(2):
                        lhsT = xat[:, p0 + half * 128 : p0 + (half + 1) * 128]
                        for jb in range(2):
                            nc.tensor.matmul(
                                out=ps[
                                    :,
                                    half * MK + jb * 512 : half * MK + (jb + 1) * 512,
                                ],
                                lhsT=lhsT,
                                rhs=kat[:, jb * 512 : (jb + 1) * 512],
                                start=True,
                                stop=True,
                            )
                    vals = valsp.tile([128, 2 * MK], bf16)
                    nc.scalar.activation(out=vals[:], in_=ps[:], func=EXP)
                    # 16 -> 1 grouped sum as a pairwise add tree.  All-bf16
                    # packed operands hit the DVE 2x perf mode (the plain
                    # tensor_reduce runs mode-less at 1x and was the
                    # kernel-wide critical path at 2.2us/tile).
                    t1 = tre1p.tile([128, 1024], bf16)
                    t2 = tre2p.tile([128, 512], bf16)
                    t3 = tre3p.tile([128, 256], bf16)
                    ot = outp.tile([128, 2 * M_KERN], f32)
                    v3 = vals[:].rearrange("p (g l) -> p g l", l=16)
                    a3 = t1[:].rearrange("p (g l) -> p g l", l=8)
                    b3 = t2[:].rearrange("p (g l) -> p g l", l=4)
                    c3 = t3[:].rearrange("p (g l) -> p g l", l=2)
                    with nc.allow_low_precision(reason="bf16 16->1 sum, tol 2e-2"):
                        nc.vector.tensor_add(a3[:], v3[:, :, 0:8], v3[:, :, 8:16])
                        nc.vector.tensor_add(b3[:], a3[:, :, 0:4], a3[:, :, 4:8])
                        nc.vector.tensor_add(c3[:], b3[:, :, 0:2], b3[:, :, 2:4])
                        nc.vector.tensor_add(
                            ot[:].rearrange("p (g l) -> p g l", l=1),
                            c3[:, :, 0:1],
                            c3[:, :, 1:2],
                        )
                    nc.sync.dma_start(
                        out=out[g0 : g0 + 128, :], in_=ot[:, 0:M_KERN]
                    )
                    nc.sync.dma_start(
                        out=out[g0 + 128 : g0 + 256, :], in_=ot[:, M_KERN : 2 * M_KERN]
                    )
    return nc


def _split_bf16(a32):
    """fp32 array -> (hi, lo) bf16 pair with hi + lo ~= a32."""
    import ml_dtypes

    hi = a32.astype(ml_dtypes.bfloat16)
    lo = (a32 - hi.astype(np.float32)).astype(ml_dtypes.bfloat16)
    return hi, lo


def _prep_operands(normal, kern):
    """Build the rank-13 augmented bf16 operands so that
    (xa.T @ ka)[i, j] ~= 2 x_i.k_j - |x_i|^2 - |k_j|^2 = -d2[i, j]."""
    import ml_dtypes

    x = np.ascontiguousarray(np.asarray(normal, dtype=np.float32))  # (n, 3)
    kf = np.asarray(kern, dtype=np.float32).reshape(MK, 3)  # (1024, 3)

    n2 = (x * x).sum(axis=1)  # (n,)
    k2 = (kf * kf).sum(axis=1)  # (1024,)

    xhi, xlo = _split_bf16(x)
    khi, klo = _split_bf16(kf)
    n2hi, n2lo = _split_bf16(n2)
    k2hi, k2lo = _split_bf16(k2)

    n = x.shape[0]
    ones_n = np.ones(n, dtype=ml_dtypes.bfloat16)
    ones_k = np.ones(MK, dtype=ml_dtypes.bfloat16)
    two_khi = (2.0 * khi.astype(np.float32)).astype(ml_dtypes.bfloat16)  # exact
    two_klo = (2.0 * klo.astype(np.float32)).astype(ml_dtypes.bfloat16)  # exact

    # row r of xa pairs with row r of ka; sum over the 15 rows gives
    # -d2 - ln(128), so a plain exp on device yields exp(-d2)/128.
    ln128 = math.log(128.0)
    ln128hi = np.float32(np.asarray(ln128, np.float32).astype(ml_dtypes.bfloat16))
    ln128lo = np.float32(ln128) - ln128hi

    xa = np.empty((N_ROWS, n), dtype=ml_dtypes.bfloat16)
    ka = np.empty((N_ROWS, MK), dtype=ml_dtypes.bfloat16)
    xa[0:3] = xhi.T
    ka[0:3] = two_khi.T
    xa[3:6] = xhi.T
    ka[3:6] = two_klo.T
    xa[6:9] = xlo.T
    ka[6:9] = two_khi.T
    xa[9] = -n2hi
    ka[9] = ones_k
    xa[10] = -n2lo
    ka[10] = ones_k
    xa[11] = ones_n
    ka[11] = -k2hi
    xa[12] = ones_n
    ka[12] = -k2lo
    xa[13] = ones_n
    ka[13] = np.full(MK, -ln128hi, dtype=ml_dtypes.bfloat16)
    xa[14] = ones_n
    ka[14] = np.full(MK, -ln128lo, dtype=ml_dtypes.bfloat16)
    return xa, ka


def kernel(normal, neighbour, kernel):  # noqa: A002 - harness-fixed names
    global _CACHED_NC, LAST_RESULTS
    from concourse.bass_utils import run_bass_kernel_spmd

    xa, ka = _prep_operands(normal, kernel)
    assert xa.shape[1] == N_TOTAL, xa.shape

    if _CACHED_NC is None:
        _CACHED_NC = _build_bass()
        if not _CACHED_NC.is_finalized():
            _CACHED_NC.finalize()

    in_maps = [
        {
            "xa": np.ascontiguousarray(xa[:, i * N_LOCAL : (i + 1) * N_LOCAL]),
            "ka": ka,
        }
        for i in range(N_CORES)
    ]
    res = run_bass_kernel_spmd(
        _CACHED_NC, in_maps, list(range(N_CORES)), trace=TRACE
    )
    LAST_RESULTS = res
    out = np.concatenate(
        [res.results[i]["out"] for i in range(N_CORES)], axis=0
    )
    return np.ascontiguousarray(out.astype(np.float32))



# revision 12
# speedup vs baseline: 1.0098x; 1.0098x over previous
"""Kernel-correlation (gnn_message_passing) Trainium2 kernel.

out[i, m] = (1/128) * sum_{l<16} exp(-||normal[i] - kernel[m, l]||^2)

Strategy (data-parallel over points, 8 NeuronCores, no collectives):
  -d2[i, j] = 2 x_i . k_j - |x_i|^2 - |k_j|^2 is a rank-13 product of two
  bf16 hi/lo-split augmented matrices (fp32-grade accuracy at full bf16 PE
  rate), computed straight into PSUM by the TensorEngine.  ScalarE then does
  exp(psum - ln 128) PSUM->SBUF in one pass (the 1/128 output scale rides the
  free activation bias), and VectorE does the grouped 16->1 reduction via a
  strided tensor_reduce.  Output rows DMA back contiguously.

Per core: 32768 points x 1024 kernel-points = 33.5M exp evals; ACT floor
~(N+352)/1.2GHz, DVE reduce floor ~N/0.96GHz -> ~280us/core expected.
"""

import math

import numpy as np

N_TOTAL = 262144
N_CORES = 8
N_LOCAL = N_TOTAL // N_CORES  # 32768
M_KERN = 64
K_SUB = 16
MK = M_KERN * K_SUB  # 1024
N_ROWS = 15  # 9 hi/lo cross terms + n2 hi/lo + k2 hi/lo + ln128 hi/lo
CHUNK_PTS = 2048  # points per input DMA chunk
ITER_PTS = 256  # points per PSUM iteration (2 tiles of 128)

TRACE = False  # set by test.py to collect a neuron profile
LAST_RESULTS = None  # BassKernelResults of the most recent run

_CACHED_NC = None


def _build_bass():
    import concourse.bacc as bacc
    import concourse.mybir as mybir
    from concourse.tile import TileContext

    f32 = mybir.dt.float32
    bf16 = mybir.dt.bfloat16
    EXP = mybir.ActivationFunctionType.Exp

    nc = bacc.Bacc()
    xa = nc.declare_dram_parameter("xa", [N_ROWS, N_LOCAL], bf16, isOutput=False)
    ka = nc.declare_dram_parameter("ka", [N_ROWS, MK], bf16, isOutput=False)
    out = nc.declare_dram_parameter("out", [N_LOCAL, M_KERN], f32, isOutput=True)

    with TileContext(nc) as tc:
        with (
            tc.tile_pool(name="kap", bufs=1) as kap,
            tc.tile_pool(name="xap", bufs=2) as xap,
            tc.tile_pool(name="valsp", bufs=2) as valsp,
            tc.tile_pool(name="tre1p", bufs=2) as tre1p,
            tc.tile_pool(name="tre2p", bufs=2) as tre2p,
            tc.tile_pool(name="tre3p", bufs=2) as tre3p,
            tc.tile_pool(name="outp", bufs=3) as outp,
            tc.tile_pool(name="psump", bufs=2, space="PSUM") as psump,
        ):
            # Two half-width kernel tiles so the first matmul only waits for
            # a 15KB DMA, not the full 30KB.
            # Warm the ACT Exp table during the initial DMA wait.
            warm = kap.tile([128, 1], f32)
            warm2 = kap.tile([128, 1], f32)
            nc.vector.memset(warm[:], 0.0)
            nc.scalar.activation(out=warm2[:], in_=warm[:], func=EXP)

            # Two half-width kernel tiles so the first matmul only waits for
            # a 15KB DMA, not the full 30KB.
            kat0 = kap.tile([N_ROWS, 512], bf16)
            kat1 = kap.tile([N_ROWS, 512], bf16)
            kats = [kat0, kat1]
            # First DMAs go out on separate queues so their issue overlaps.
            nc.gpsimd.dma_start(out=kats[0][:], in_=ka[:, 0:512])
            nc.sync.dma_start(out=kats[1][:], in_=ka[:, 512:1024])

            def emit_iter(g0, xat, p0, nhalves):
                npts = nhalves * 128
                w = nhalves * MK
                ps = psump.tile([128, 2 * MK], f32)
                for half in range(nhalves):
                    lhsT = xat[:, p0 + half * 128 : p0 + (half + 1) * 128]
                    for jb in range(2):
                        nc.tensor.matmul(
                            out=ps[
                                :, half * MK + jb * 512 : half * MK + (jb + 1) * 512
                            ],
                            lhsT=lhsT,
                            rhs=kats[jb][:],
                            start=True,
                            stop=True,
                        )
                vals = valsp.tile([128, w], bf16)
                nc.scalar.activation(out=vals[:], in_=ps[:, 0:w], func=EXP)
                # 16 -> 1 grouped sum as a pairwise add tree.  All-bf16
                # packed operands hit the DVE 2x perf mode (the plain
                # tensor_reduce runs mode-less at 1x and was the
                # kernel-wide critical path at 2.2us/tile).
                t1 = tre1p.tile([128, w // 2], bf16)
                t2 = tre2p.tile([128, w // 4], bf16)
                t3 = tre3p.tile([128, w // 8], bf16)
                ot = outp.tile([128, w // 16], f32)
                v3 = vals[:].rearrange("p (g l) -> p g l", l=16)
                a3 = t1[:].rearrange("p (g l) -> p g l", l=8)
                b3 = t2[:].rearrange("p (g l) -> p g l", l=4)
                c3 = t3[:].rearrange("p (g l) -> p g l", l=2)
                with nc.allow_low_precision(reason="bf16 16->1 sum, tol 2e-2"):
                    nc.vector.tensor_add(a3[:], v3[:, :, 0:8], v3[:, :, 8:16])
                    nc.vector.tensor_add(b3[:], a3[:, :, 0:4], a3[:, :, 4:8])
                    nc.vector.tensor_add(c3[:], b3[:, :, 0:2], b3[:, :, 2:4])
                    nc.vector.tensor_add(
                        ot[:].rearrange("p (g l) -> p g l", l=1),
                        c3[:, :, 0:1],
                        c3[:, :, 1:2],
                    )
                for half in range(nhalves):
                    nc.sync.dma_start(
                        out=out[g0 + half * 128 : g0 + (half + 1) * 128, :],
                        in_=ot[:, half * M_KERN : (half + 1) * M_KERN],
                    )

            # Peel a small 256-pt first chunk so compute starts ~3us sooner,
            # stream the middle in 2048-pt chunks, and end with two 128-pt
            # iterations so the pipeline tail drains faster.
            chunks = [(0, 256)]
            off = 256
            while off < N_LOCAL:
                sz = min(CHUNK_PTS, N_LOCAL - off)
                chunks.append((off, sz))
                off += sz
            first = True
            for c0, csz in chunks:
                xat = xap.tile([N_ROWS, csz], bf16)
                if first:
                    nc.sync.dma_start(out=xat[:], in_=xa[:, c0 : c0 + csz])
                else:
                    nc.gpsimd.dma_start(out=xat[:], in_=xa[:, c0 : c0 + csz])
                last_chunk = c0 + csz == N_LOCAL
                n_iters = csz // ITER_PTS
                for it in range(n_iters):
                    p0 = it * ITER_PTS
                    g0 = c0 + p0
                    if (last_chunk and it == n_iters - 1) or (first and it == 0):
                        emit_iter(g0, xat, p0, 1)
                        emit_iter(g0 + 128, xat, p0 + 128, 1)
                    else:
                        emit_iter(g0, xat, p0, 2)
                first = False
    return nc


def _split_bf16(a32):
    """fp32 array -> (hi, lo) bf16 pair with hi + lo ~= a32."""
    import ml_dtypes

    hi = a32.astype(ml_dtypes.bfloat16)
    lo = (a32 - hi.astype(np.float32)).astype(ml_dtypes.bfloat16)
    return hi, lo


def _prep_operands(normal, kern):
    """Build the rank-13 augmented bf16 operands so that
    (xa.T @ ka)[i, j] ~= 2 x_i.k_j - |x_i|^2 - |k_j|^2 = -d2[i, j]."""
    import ml_dtypes

    x = np.ascontiguousarray(np.asarray(normal, dtype=np.float32))  # (n, 3)
    kf = np.asarray(kern, dtype=np.float32).reshape(MK, 3)  # (1024, 3)

    n2 = (x * x).sum(axis=1)  # (n,)
    k2 = (kf * kf).sum(axis=1)  # (1024,)

    xhi, xlo = _split_bf16(x)
    khi, klo = _split_bf16(kf)
    n2hi, n2lo = _split_bf16(n2)
    k2hi, k2lo = _split_bf16(k2)

    n = x.shape[0]
    ones_n = np.ones(n, dtype=ml_dtypes.bfloat16)
    ones_k = np.ones(MK, dtype=ml_dtypes.bfloat16)
    two_khi = (2.0 * khi.astype(np.float32)).astype(ml_dtypes.bfloat16)  # exact
    two_klo = (2.0 * klo.astype(np.float32)).astype(ml_dtypes.bfloat16)  # exact

    # row r of xa pairs with row r of ka; sum over the 15 rows gives
    # -d2 - ln(128), so a plain exp on device yields exp(-d2)/128.
    ln128 = math.log(128.0)
    ln128hi = np.float32(np.asarray(ln128, np.float32).astype(ml_dtypes.bfloat16))
    ln128lo = np.float32(ln128) - ln128hi

    xa = np.empty((N_ROWS, n), dtype=ml_dtypes.bfloat16)
    ka = np.empty((N_ROWS, MK), dtype=ml_dtypes.bfloat16)
    xa[0:3] = xhi.T
    ka[0:3] = two_khi.T
    xa[3:6] = xhi.T
    ka[3:6] = two_klo.T
    xa[6:9] = xlo.T
    ka[6:9] = two_khi.T
    xa[9] = -n2hi
    ka[9] = ones_k
    xa[10] = -n2lo
    ka[10] = ones_k
    xa[11] = ones_n
    ka[11] = -k2hi
    xa[12] = ones_n
    ka[12] = -k2lo
    xa[13] = ones_n
    ka[13] = np.full(MK, -ln128hi, dtype=ml_dtypes.bfloat16)
    xa[14] = ones_n
    ka[14] = np.full(MK, -ln128lo, dtype=ml_dtypes.bfloat16)
    return xa, ka


def kernel(normal, neighbour, kernel):  # noqa: A002 - harness-fixed names
    global _CACHED_NC, LAST_RESULTS
    from concourse.bass_utils import run_bass_kernel_spmd

    xa, ka = _prep_operands(normal, kernel)
    assert xa.shape[1] == N_TOTAL, xa.shape

    if _CACHED_NC is None:
        _CACHED_NC = _build_bass()
        if not _CACHED_NC.is_finalized():
            _CACHED_NC.finalize()

    in_maps = [
        {
            "xa": np.ascontiguousarray(xa[:, i * N_LOCAL : (i + 1) * N_LOCAL]),
            "ka": ka,
        }
        for i in range(N_CORES)
    ]
    res = run_bass_kernel_spmd(
        _CACHED_NC, in_maps, list(range(N_CORES)), trace=TRACE
    )
    LAST_RESULTS = res
    out = np.concatenate(
        [res.results[i]["out"] for i in range(N_CORES)], axis=0
    )
    return np.ascontiguousarray(out.astype(np.float32))



# revision 22
# speedup vs baseline: 1.0098x; 1.0000x over previous
"""Kernel-correlation (gnn_message_passing) Trainium2 kernel.

out[i, m] = (1/128) * sum_{l<16} exp(-||normal[i] - kernel[m, l]||^2)

Strategy (data-parallel over points, 8 NeuronCores, no collectives):
  -d2[i, j] = 2 x_i . k_j - |x_i|^2 - |k_j|^2 is a rank-13 product of two
  bf16 hi/lo-split augmented matrices (fp32-grade accuracy at full bf16 PE
  rate), computed straight into PSUM by the TensorEngine.  ScalarE then does
  exp(psum - ln 128) PSUM->SBUF bf16 in one pass (the 1/128 output scale
  rides the per-partition activation bias), and VectorE does the grouped
  16->1 reduction as a 4-level pairwise bf16 add tree (packed 2-byte
  operands engage the DVE 2x perf mode; the plain tensor_reduce runs
  mode-less at 1x and was the original critical path).  Output rows DMA
  back contiguously.

Per core: 32768 points x 1024 kernel-points = 33.5M exp evals.  ACT is the
bound engine: 128 instrs x (2048+222)*0.833ns = 242.6us busy; DVE tree
~168us, PE ~114us.  Timeline-sim: 253.7us (was 300.1us with tensor_reduce).
"""

import math

import numpy as np

N_TOTAL = 262144
N_CORES = 8
N_LOCAL = N_TOTAL // N_CORES  # 32768
M_KERN = 64
K_SUB = 16
MK = M_KERN * K_SUB  # 1024
N_ROWS = 13  # 9 hi/lo cross terms + n2 hi/lo + k2 hi/lo (ln128 rides ACT bias)
CHUNK_PTS = 2048  # points per input DMA chunk
ITER_PTS = 256  # points per PSUM iteration (2 tiles of 128)
NEG_LN128 = -float(math.log(128.0))  # exp output scale, applied as ACT bias

TRACE = False  # set by test.py to collect a neuron profile
LAST_RESULTS = None  # BassKernelResults of the most recent run

_CACHED_NC = None


def _build_bass():
    import concourse.bacc as bacc
    import concourse.mybir as mybir
    from concourse.tile import TileContext

    f32 = mybir.dt.float32
    bf16 = mybir.dt.bfloat16
    EXP = mybir.ActivationFunctionType.Exp

    nc = bacc.Bacc()
    xa = nc.declare_dram_parameter("xa", [N_ROWS, N_LOCAL], bf16, isOutput=False)
    ka = nc.declare_dram_parameter("ka", [N_ROWS, MK], bf16, isOutput=False)
    out = nc.declare_dram_parameter("out", [N_LOCAL, M_KERN], f32, isOutput=True)

    with TileContext(nc) as tc:
        with (
            tc.tile_pool(name="kap", bufs=1) as kap,
            tc.tile_pool(name="xap", bufs=2) as xap,
            tc.tile_pool(name="valsp", bufs=2) as valsp,
            tc.tile_pool(name="tre1p", bufs=2) as tre1p,
            tc.tile_pool(name="tre2p", bufs=2) as tre2p,
            tc.tile_pool(name="tre3p", bufs=2) as tre3p,
            tc.tile_pool(name="outp", bufs=3) as outp,
            tc.tile_pool(name="psump", bufs=2, space="PSUM") as psump,
        ):
            # Per-partition ACT bias tile holding -ln(128); doubles as the
            # warm-up input so the Exp table loads during the initial DMAs.
            biast = kap.tile([128, 1], f32)
            warm2 = kap.tile([128, 1], f32)
            nc.vector.memset(biast[:], NEG_LN128)
            nc.scalar.activation(out=warm2[:], in_=biast[:], func=EXP)

            # Two half-width kernel tiles so the first matmul only waits for
            # a 15KB DMA, not the full 30KB.
            kat0 = kap.tile([N_ROWS, 512], bf16)
            kat1 = kap.tile([N_ROWS, 512], bf16)
            kats = [kat0, kat1]
            # First DMAs go out on separate queues so their issue overlaps.
            nc.gpsimd.dma_start(out=kats[0][:], in_=ka[:, 0:512])
            nc.sync.dma_start(out=kats[1][:], in_=ka[:, 512:1024])

            def emit_iter(g0, xat, p0, nhalves):
                w = nhalves * MK
                ps = psump.tile([128, 2 * MK], f32)
                for half in range(nhalves):
                    lhsT = xat[:, p0 + half * 128 : p0 + (half + 1) * 128]
                    for jb in range(2):
                        nc.tensor.matmul(
                            out=ps[
                                :, half * MK + jb * 512 : half * MK + (jb + 1) * 512
                            ],
                            lhsT=lhsT,
                            rhs=kats[jb][:],
                            start=True,
                            stop=True,
                        )
                vals = valsp.tile([128, w], bf16)
                nc.scalar.activation(
                    out=vals[:], in_=ps[:, 0:w], func=EXP, bias=biast[:]
                )
                # 16 -> 1 grouped sum as a pairwise add tree.  All-bf16
                # packed operands hit the DVE 2x perf mode (the plain
                # tensor_reduce runs mode-less at 1x and was the
                # kernel-wide critical path at 2.2us/tile).
                t1 = tre1p.tile([128, w // 2], bf16)
                t2 = tre2p.tile([128, w // 4], bf16)
                t3 = tre3p.tile([128, w // 8], bf16)
                ot = outp.tile([128, w // 16], f32)
                v3 = vals[:].rearrange("p (g l) -> p g l", l=16)
                a3 = t1[:].rearrange("p (g l) -> p g l", l=8)
                b3 = t2[:].rearrange("p (g l) -> p g l", l=4)
                c3 = t3[:].rearrange("p (g l) -> p g l", l=2)
                with nc.allow_low_precision(reason="bf16 16->1 sum, tol 2e-2"):
                    nc.vector.tensor_add(a3[:], v3[:, :, 0:8], v3[:, :, 8:16])
                    nc.vector.tensor_add(b3[:], a3[:, :, 0:4], a3[:, :, 4:8])
                    nc.vector.tensor_add(c3[:], b3[:, :, 0:2], b3[:, :, 2:4])
                    nc.vector.tensor_add(
                        ot[:].rearrange("p (g l) -> p g l", l=1),
                        c3[:, :, 0:1],
                        c3[:, :, 1:2],
                    )
                for half in range(nhalves):
                    nc.sync.dma_start(
                        out=out[g0 + half * 128 : g0 + (half + 1) * 128, :],
                        in_=ot[:, half * M_KERN : (half + 1) * M_KERN],
                    )

            # Ramp the chunk schedule (256, 1024, then 2048s) so compute
            # starts ~3us sooner and the second chunk lands before the first
            # runs dry; end with two 128-pt iterations so the tail drains
            # faster.
            chunks = [(0, 256), (256, 1024)]
            off = 1280
            while off < N_LOCAL:
                sz = min(CHUNK_PTS, N_LOCAL - off)
                chunks.append((off, sz))
                off += sz
            first = True
            for c0, csz in chunks:
                xat = xap.tile([N_ROWS, csz], bf16)
                if first:
                    nc.sync.dma_start(out=xat[:], in_=xa[:, c0 : c0 + csz])
                else:
                    nc.gpsimd.dma_start(out=xat[:], in_=xa[:, c0 : c0 + csz])
                last_chunk = c0 + csz == N_LOCAL
                n_iters = csz // ITER_PTS
                for it in range(n_iters):
                    p0 = it * ITER_PTS
                    g0 = c0 + p0
                    if (last_chunk and it == n_iters - 1) or (first and it == 0):
                        emit_iter(g0, xat, p0, 1)
                        emit_iter(g0 + 128, xat, p0 + 128, 1)
                    else:
                        emit_iter(g0, xat, p0, 2)
                first = False
    return nc


def _split_bf16(a32):
    """fp32 array -> (hi, lo) bf16 pair with hi + lo ~= a32."""
    import ml_dtypes

    hi = a32.astype(ml_dtypes.bfloat16)
    lo = (a32 - hi.astype(np.float32)).astype(ml_dtypes.bfloat16)
    return hi, lo


def _prep_operands(normal, kern):
    """Build the rank-13 augmented bf16 operands so that
    (xa.T @ ka)[i, j] ~= 2 x_i.k_j - |x_i|^2 - |k_j|^2 = -d2[i, j]."""
    import ml_dtypes

    x = np.ascontiguousarray(np.asarray(normal, dtype=np.float32))  # (n, 3)
    kf = np.asarray(kern, dtype=np.float32).reshape(MK, 3)  # (1024, 3)

    n2 = (x * x).sum(axis=1)  # (n,)
    k2 = (kf * kf).sum(axis=1)  # (1024,)

    xhi, xlo = _split_bf16(x)
    khi, klo = _split_bf16(kf)
    n2hi, n2lo = _split_bf16(n2)
    k2hi, k2lo = _split_bf16(k2)

    n = x.shape[0]
    ones_n = np.ones(n, dtype=ml_dtypes.bfloat16)
    ones_k = np.ones(MK, dtype=ml_dtypes.bfloat16)
    two_khi = (2.0 * khi.astype(np.float32)).astype(ml_dtypes.bfloat16)  # exact
    two_klo = (2.0 * klo.astype(np.float32)).astype(ml_dtypes.bfloat16)  # exact

    # row r of xa pairs with row r of ka; sum over the 13 rows gives -d2;
    # the -ln(128) output scale rides the ACT bias, so the device exp
    # yields exp(-d2)/128.
    xa = np.empty((N_ROWS, n), dtype=ml_dtypes.bfloat16)
    ka = np.empty((N_ROWS, MK), dtype=ml_dtypes.bfloat16)
    xa[0:3] = xhi.T
    ka[0:3] = two_khi.T
    xa[3:6] = xhi.T
    ka[3:6] = two_klo.T
    xa[6:9] = xlo.T
    ka[6:9] = two_khi.T
    xa[9] = -n2hi
    ka[9] = ones_k
    xa[10] = -n2lo
    ka[10] = ones_k
    xa[11] = ones_n
    ka[11] = -k2hi
    xa[12] = ones_n
    ka[12] = -k2lo
    return xa, ka


def kernel(normal, neighbour, kernel):  # noqa: A002 - harness-fixed names
    global _CACHED_NC, LAST_RESULTS
    from concourse.bass_utils import run_bass_kernel_spmd

    xa, ka = _prep_operands(normal, kernel)
    assert xa.shape[1] == N_TOTAL, xa.shape

    if _CACHED_NC is None:
        _CACHED_NC = _build_bass()
        if not _CACHED_NC.is_finalized():
            _CACHED_NC.finalize()

    in_maps = [
        {
            "xa": np.ascontiguousarray(xa[:, i * N_LOCAL : (i + 1) * N_LOCAL]),
            "ka": ka,
        }
        for i in range(N_CORES)
    ]
    res = run_bass_kernel_spmd(
        _CACHED_NC, in_maps, list(range(N_CORES)), trace=TRACE
    )
    LAST_RESULTS = res
    out = np.concatenate(
        [res.results[i]["out"] for i in range(N_CORES)], axis=0
    )
    return np.ascontiguousarray(out.astype(np.float32))



# revision 34
# speedup vs baseline: 1.0142x; 1.0044x over previous
"""Kernel-correlation (gnn_message_passing) Trainium2 kernel.

out[i, m] = (1/128) * sum_{l<16} exp(-||normal[i] - kernel[m, l]||^2)

Strategy (data-parallel over points, 8 NeuronCores, no collectives):
  -d2[i, j] = 2 x_i . k_j - |x_i|^2 - |k_j|^2 is a rank-13 product of two
  bf16 hi/lo-split augmented matrices (fp32-grade accuracy at full bf16 PE
  rate), computed straight into PSUM by the TensorEngine.  ScalarE then does
  exp(psum - ln 128) PSUM->SBUF bf16 in one pass (the 1/128 output scale
  rides the per-partition activation bias), and VectorE does the grouped
  16->1 reduction as a 4-level pairwise bf16 add tree (packed 2-byte
  operands engage the DVE 2x perf mode; the plain tensor_reduce runs
  mode-less at 1x and was the original critical path).  Output rows DMA
  back contiguously.

Per core: 32768 points x 1024 kernel-points = 33.5M exp evals.  ACT is the
bound engine: 128 instrs x (2048+222)*0.833ns = 242.6us busy (~96% of the
kernel span); DVE tree ~168us, PE ~114us.  Pipeline head/tail trimmed via a
ramped chunk schedule, split kernel-tile DMAs on separate queues, an ACT
Exp-table warm-up, and PE p-state pre-warm matmuls.  Timeline-sim: 252.6us
(was 300.1us with the mode-less tensor_reduce on the critical path).
"""

import math

import numpy as np

N_TOTAL = 262144
N_CORES = 8
N_LOCAL = N_TOTAL // N_CORES  # 32768
M_KERN = 64
K_SUB = 16
MK = M_KERN * K_SUB  # 1024
N_ROWS = 13  # 9 hi/lo cross terms + n2 hi/lo + k2 hi/lo (ln128 rides ACT bias)
CHUNK_PTS = 2048  # points per input DMA chunk
ITER_PTS = 256  # points per PSUM iteration (2 tiles of 128)
NEG_LN128 = -float(math.log(128.0))  # exp output scale, applied as ACT bias

TRACE = False  # set by test.py to collect a neuron profile
LAST_RESULTS = None  # BassKernelResults of the most recent run

_CACHED_NC = None


def _build_bass():
    import concourse.bacc as bacc
    import concourse.mybir as mybir
    from concourse.tile import TileContext

    f32 = mybir.dt.float32
    bf16 = mybir.dt.bfloat16
    EXP = mybir.ActivationFunctionType.Exp

    nc = bacc.Bacc()
    xa = nc.declare_dram_parameter("xa", [N_ROWS, N_LOCAL], bf16, isOutput=False)
    ka = nc.declare_dram_parameter("ka", [N_ROWS, MK], bf16, isOutput=False)
    out = nc.declare_dram_parameter("out", [N_LOCAL, M_KERN], f32, isOutput=True)

    with TileContext(nc) as tc:
        with (
            tc.tile_pool(name="kap", bufs=1) as kap,
            tc.tile_pool(name="xap", bufs=2) as xap,
            tc.tile_pool(name="valsp", bufs=2) as valsp,
            tc.tile_pool(name="tre1p", bufs=2) as tre1p,
            tc.tile_pool(name="tre2p", bufs=2) as tre2p,
            tc.tile_pool(name="tre3p", bufs=2) as tre3p,
            tc.tile_pool(name="outp", bufs=3) as outp,
            tc.tile_pool(name="psump", bufs=2, space="PSUM") as psump,
        ):
            # Per-partition ACT bias tile holding -ln(128); doubles as the
            # warm-up input so the Exp table loads during the initial DMAs.
            biast = kap.tile([128, 1], f32)
            warm2 = kap.tile([128, 1], f32)
            nc.vector.memset(biast[:], NEG_LN128)
            nc.scalar.activation(out=warm2[:], in_=biast[:], func=EXP)

            # Pre-warm the PE p-state with dummy matmuls on scratch data so
            # the first real iterations run at full clock (the cold PE runs
            # ~2-4x slower and was stalling the ACT stream by ~2us).
            scr_l = kap.tile([N_ROWS, 128], bf16)
            scr_r = kap.tile([N_ROWS, 512], bf16)
            nc.vector.memset(scr_l[:], 0.0)
            nc.vector.memset(scr_r[:], 0.0)
            # Two half-width kernel tiles so the first matmul only waits for
            # a 15KB DMA, not the full 30KB.
            kat0 = kap.tile([N_ROWS, 512], bf16)
            kat1 = kap.tile([N_ROWS, 512], bf16)
            kats = [kat0, kat1]
            # First DMAs go out on separate queues so their issue overlaps.
            nc.gpsimd.dma_start(out=kats[0][:], in_=ka[:, 0:512])
            nc.sync.dma_start(out=kats[1][:], in_=ka[:, 512:1024])

            def emit_iter(g0, xat, p0, nhalves, warm=False):
                w = nhalves * MK
                ps = psump.tile([128, 2 * MK], f32)
                if warm:
                    # The first (128-pt) iteration only uses ps[:, 0:MK];
                    # run dummy matmuls into the unused half while the first
                    # DMAs are in flight so the PE p-state ramps up before
                    # real data arrives.
                    for _ in range(4):
                        nc.tensor.matmul(
                            out=ps[:, MK : MK + 512],
                            lhsT=scr_l[:],
                            rhs=scr_r[:],
                            start=True,
                            stop=True,
                        )
                for half in range(nhalves):
                    lhsT = xat[:, p0 + half * 128 : p0 + (half + 1) * 128]
                    for jb in range(2):
                        nc.tensor.matmul(
                            out=ps[
                                :, half * MK + jb * 512 : half * MK + (jb + 1) * 512
                            ],
                            lhsT=lhsT,
                            rhs=kats[jb][:],
                            start=True,
                            stop=True,
                        )
                vals = valsp.tile([128, w], bf16)
                nc.scalar.activation(
                    out=vals[:], in_=ps[:, 0:w], func=EXP, bias=biast[:]
                )
                # 16 -> 1 grouped sum as a pairwise add tree.  All-bf16
                # packed operands hit the DVE 2x perf mode (the plain
                # tensor_reduce runs mode-less at 1x and was the
                # kernel-wide critical path at 2.2us/tile).
                t1 = tre1p.tile([128, w // 2], bf16)
                t2 = tre2p.tile([128, w // 4], bf16)
                t3 = tre3p.tile([128, w // 8], bf16)
                ot = outp.tile([128, w // 16], f32)
                v3 = vals[:].rearrange("p (g l) -> p g l", l=16)
                a3 = t1[:].rearrange("p (g l) -> p g l", l=8)
                b3 = t2[:].rearrange("p (g l) -> p g l", l=4)
                c3 = t3[:].rearrange("p (g l) -> p g l", l=2)
                with nc.allow_low_precision(reason="bf16 16->1 sum, tol 2e-2"):
                    nc.vector.tensor_add(a3[:], v3[:, :, 0:8], v3[:, :, 8:16])
                    nc.vector.tensor_add(b3[:], a3[:, :, 0:4], a3[:, :, 4:8])
                    nc.vector.tensor_add(c3[:], b3[:, :, 0:2], b3[:, :, 2:4])
                    nc.vector.tensor_add(
                        ot[:].rearrange("p (g l) -> p g l", l=1),
                        c3[:, :, 0:1],
                        c3[:, :, 1:2],
                    )
                for half in range(nhalves):
                    nc.sync.dma_start(
                        out=out[g0 + half * 128 : g0 + (half + 1) * 128, :],
                        in_=ot[:, half * M_KERN : (half + 1) * M_KERN],
                    )

            # Ramp the chunk schedule (256, 1024, then 2048s) so compute
            # starts ~3us sooner and the second chunk lands before the first
            # runs dry; end with two 128-pt iterations so the tail drains
            # faster.
            chunks = [(0, 256), (256, 1024)]
            off = 1280
            while off < N_LOCAL:
                sz = min(CHUNK_PTS, N_LOCAL - off)
                chunks.append((off, sz))
                off += sz
            first = True
            for c0, csz in chunks:
                xat = xap.tile([N_ROWS, csz], bf16)
                if first:
                    nc.sync.dma_start(out=xat[:], in_=xa[:, c0 : c0 + csz])
                else:
                    nc.gpsimd.dma_start(out=xat[:], in_=xa[:, c0 : c0 + csz])
                last_chunk = c0 + csz == N_LOCAL
                n_iters = csz // ITER_PTS
                for it in range(n_iters):
                    p0 = it * ITER_PTS
                    g0 = c0 + p0
                    if (last_chunk and it == n_iters - 1) or (first and it == 0):
                        emit_iter(g0, xat, p0, 1, warm=first)
                        emit_iter(g0 + 128, xat, p0 + 128, 1)
                    else:
                        emit_iter(g0, xat, p0, 2)
                first = False
    return nc


def _split_bf16(a32):
    """fp32 array -> (hi, lo) bf16 pair with hi + lo ~= a32."""
    import ml_dtypes

    hi = a32.astype(ml_dtypes.bfloat16)
    lo = (a32 - hi.astype(np.float32)).astype(ml_dtypes.bfloat16)
    return hi, lo


def _prep_operands(normal, kern):
    """Build the rank-13 augmented bf16 operands so that
    (xa.T @ ka)[i, j] ~= 2 x_i.k_j - |x_i|^2 - |k_j|^2 = -d2[i, j]."""
    import ml_dtypes

    x = np.ascontiguousarray(np.asarray(normal, dtype=np.float32))  # (n, 3)
    kf = np.asarray(kern, dtype=np.float32).reshape(MK, 3)  # (1024, 3)

    n2 = (x * x).sum(axis=1)  # (n,)
    k2 = (kf * kf).sum(axis=1)  # (1024,)

    xhi, xlo = _split_bf16(x)
    khi, klo = _split_bf16(kf)
    n2hi, n2lo = _split_bf16(n2)
    k2hi, k2lo = _split_bf16(k2)

    n = x.shape[0]
    ones_n = np.ones(n, dtype=ml_dtypes.bfloat16)
    ones_k = np.ones(MK, dtype=ml_dtypes.bfloat16)
    two_khi = (2.0 * khi.astype(np.float32)).astype(ml_dtypes.bfloat16)  # exact
    two_klo = (2.0 * klo.astype(np.float32)).astype(ml_dtypes.bfloat16)  # exact

    # row r of xa pairs with row r of ka; sum over the 13 rows gives -d2;
    # the -ln(128) output scale rides the ACT bias, so the device exp
    # yields exp(-d2)/128.
    xa = np.empty((N_ROWS, n), dtype=ml_dtypes.bfloat16)
    ka = np.empty((N_ROWS, MK), dtype=ml_dtypes.bfloat16)
    xa[0:3] = xhi.T
    ka[0:3] = two_khi.T
    xa[3:6] = xhi.T
    ka[3:6] = two_klo.T
    xa[6:9] = xlo.T
    ka[6:9] = two_khi.T
    xa[9] = -n2hi
    ka[9] = ones_k
    xa[10] = -n2lo
    ka[10] = ones_k
    xa[11] = ones_n
    ka[11] = -k2hi
    xa[12] = ones_n
    ka[12] = -k2lo
    return xa, ka


def kernel(normal, neighbour, kernel):  # noqa: A002 - harness-fixed names
    global _CACHED_NC, LAST_RESULTS
    from concourse.bass_utils import run_bass_kernel_spmd

    xa, ka = _prep_operands(normal, kernel)
    assert xa.shape[1] == N_TOTAL, xa.shape

    if _CACHED_NC is None:
        _CACHED_NC = _build_bass()
        if not _CACHED_NC.is_finalized():
            _CACHED_NC.finalize()

    in_maps = [
        {
            "xa": np.ascontiguousarray(xa[:, i * N_LOCAL : (i + 1) * N_LOCAL]),
            "ka": ka,
        }
        for i in range(N_CORES)
    ]
    res = run_bass_kernel_spmd(
        _CACHED_NC, in_maps, list(range(N_CORES)), trace=TRACE
    )
    LAST_RESULTS = res
    out = np.concatenate(
        [res.results[i]["out"] for i in range(N_CORES)], axis=0
    )
    return np.ascontiguousarray(out.astype(np.float32))



# revision 36
# speedup vs baseline: 1.0145x; 1.0003x over previous
"""Kernel-correlation (gnn_message_passing) Trainium2 kernel.

out[i, m] = (1/128) * sum_{l<16} exp(-||normal[i] - kernel[m, l]||^2)

Strategy (data-parallel over points, 8 NeuronCores, no collectives):
  -d2[i, j] = 2 x_i . k_j - |x_i|^2 - |k_j|^2 is a rank-13 product of two
  bf16 hi/lo-split augmented matrices (fp32-grade accuracy at full bf16 PE
  rate), computed straight into PSUM by the TensorEngine.  ScalarE then does
  exp(psum - ln 128) PSUM->SBUF bf16 in one pass (the 1/128 output scale
  rides the per-partition activation bias), and VectorE does the grouped
  16->1 reduction as a 4-level pairwise bf16 add tree (packed 2-byte
  operands engage the DVE 2x perf mode; the plain tensor_reduce runs
  mode-less at 1x and was the original critical path).  Output rows DMA
  back contiguously.

Per core: 32768 points x 1024 kernel-points = 33.5M exp evals.  ACT is the
bound engine: 128 instrs x (2048+222)*0.833ns = 242.6us busy (~96% of the
kernel span); DVE tree ~168us, PE ~114us.  Pipeline head/tail trimmed via a
ramped chunk schedule, split kernel-tile DMAs on separate queues, an ACT
Exp-table warm-up, and PE p-state pre-warm matmuls.  Timeline-sim: 252.6us
(was 300.1us with the mode-less tensor_reduce on the critical path).
"""

import math

import numpy as np

N_TOTAL = 262144
N_CORES = 8
N_LOCAL = N_TOTAL // N_CORES  # 32768
M_KERN = 64
K_SUB = 16
MK = M_KERN * K_SUB  # 1024
N_ROWS = 13  # 9 hi/lo cross terms + n2 hi/lo + k2 hi/lo (ln128 rides ACT bias)
CHUNK_PTS = 2048  # points per input DMA chunk
ITER_PTS = 256  # points per PSUM iteration (2 tiles of 128)
NEG_LN128 = -float(math.log(128.0))  # exp output scale, applied as ACT bias

TRACE = False  # set by test.py to collect a neuron profile
LAST_RESULTS = None  # BassKernelResults of the most recent run

_CACHED_NC = None


def _build_bass():
    import concourse.bacc as bacc
    import concourse.mybir as mybir
    from concourse.tile import TileContext

    f32 = mybir.dt.float32
    bf16 = mybir.dt.bfloat16
    EXP = mybir.ActivationFunctionType.Exp

    nc = bacc.Bacc()
    xa = nc.declare_dram_parameter("xa", [N_ROWS, N_LOCAL], bf16, isOutput=False)
    ka = nc.declare_dram_parameter("ka", [N_ROWS, MK], bf16, isOutput=False)
    out = nc.declare_dram_parameter("out", [N_LOCAL, M_KERN], f32, isOutput=True)

    with TileContext(nc) as tc:
        with (
            tc.tile_pool(name="kap", bufs=1) as kap,
            tc.tile_pool(name="xap", bufs=3) as xap,
            tc.tile_pool(name="valsp", bufs=3) as valsp,
            tc.tile_pool(name="tre1p", bufs=3) as tre1p,
            tc.tile_pool(name="tre2p", bufs=3) as tre2p,
            tc.tile_pool(name="tre3p", bufs=3) as tre3p,
            tc.tile_pool(name="outp", bufs=4) as outp,
            tc.tile_pool(name="psump", bufs=2, space="PSUM") as psump,
        ):
            # Per-partition ACT bias tile holding -ln(128); doubles as the
            # warm-up input so the Exp table loads during the initial DMAs.
            biast = kap.tile([128, 1], f32)
            warm2 = kap.tile([128, 1], f32)
            nc.vector.memset(biast[:], NEG_LN128)
            nc.scalar.activation(out=warm2[:], in_=biast[:], func=EXP)

            # Pre-warm the PE p-state with dummy matmuls on scratch data so
            # the first real iterations run at full clock (the cold PE runs
            # ~2-4x slower and was stalling the ACT stream by ~2us).
            scr_l = kap.tile([N_ROWS, 128], bf16)
            scr_r = kap.tile([N_ROWS, 512], bf16)
            nc.vector.memset(scr_l[:], 0.0)
            nc.vector.memset(scr_r[:], 0.0)
            # Two half-width kernel tiles so the first matmul only waits for
            # a 15KB DMA, not the full 30KB.
            kat0 = kap.tile([N_ROWS, 512], bf16)
            kat1 = kap.tile([N_ROWS, 512], bf16)
            kats = [kat0, kat1]
            # First DMAs go out on separate queues so their issue overlaps.
            nc.gpsimd.dma_start(out=kats[0][:], in_=ka[:, 0:512])
            nc.sync.dma_start(out=kats[1][:], in_=ka[:, 512:1024])

            def emit_iter(g0, xat, p0, nhalves, warm=False):
                w = nhalves * MK
                ps = psump.tile([128, 2 * MK], f32)
                if warm:
                    # The first (128-pt) iteration only uses ps[:, 0:MK];
                    # run dummy matmuls into the unused half while the first
                    # DMAs are in flight so the PE p-state ramps up before
                    # real data arrives.
                    for _ in range(4):
                        nc.tensor.matmul(
                            out=ps[:, MK : MK + 512],
                            lhsT=scr_l[:],
                            rhs=scr_r[:],
                            start=True,
                            stop=True,
                        )
                for half in range(nhalves):
                    lhsT = xat[:, p0 + half * 128 : p0 + (half + 1) * 128]
                    for jb in range(2):
                        nc.tensor.matmul(
                            out=ps[
                                :, half * MK + jb * 512 : half * MK + (jb + 1) * 512
                            ],
                            lhsT=lhsT,
                            rhs=kats[jb][:],
                            start=True,
                            stop=True,
                        )
                vals = valsp.tile([128, w], bf16)
                nc.scalar.activation(
                    out=vals[:], in_=ps[:, 0:w], func=EXP, bias=biast[:]
                )
                # 16 -> 1 grouped sum as a pairwise add tree.  All-bf16
                # packed operands hit the DVE 2x perf mode (the plain
                # tensor_reduce runs mode-less at 1x and was the
                # kernel-wide critical path at 2.2us/tile).
                t1 = tre1p.tile([128, w // 2], bf16)
                t2 = tre2p.tile([128, w // 4], bf16)
                t3 = tre3p.tile([128, w // 8], bf16)
                ot = outp.tile([128, w // 16], f32)
                v3 = vals[:].rearrange("p (g l) -> p g l", l=16)
                a3 = t1[:].rearrange("p (g l) -> p g l", l=8)
                b3 = t2[:].rearrange("p (g l) -> p g l", l=4)
                c3 = t3[:].rearrange("p (g l) -> p g l", l=2)
                with nc.allow_low_precision(reason="bf16 16->1 sum, tol 2e-2"):
                    nc.vector.tensor_add(a3[:], v3[:, :, 0:8], v3[:, :, 8:16])
                    nc.vector.tensor_add(b3[:], a3[:, :, 0:4], a3[:, :, 4:8])
                    nc.vector.tensor_add(c3[:], b3[:, :, 0:2], b3[:, :, 2:4])
                    nc.vector.tensor_add(
                        ot[:].rearrange("p (g l) -> p g l", l=1),
                        c3[:, :, 0:1],
                        c3[:, :, 1:2],
                    )
                for half in range(nhalves):
                    nc.sync.dma_start(
                        out=out[g0 + half * 128 : g0 + (half + 1) * 128, :],
                        in_=ot[:, half * M_KERN : (half + 1) * M_KERN],
                    )

            # Ramp the chunk schedule (256, 1024, then 2048s) so compute
            # starts ~3us sooner and the second chunk lands before the first
            # runs dry; end with two 128-pt iterations so the tail drains
            # faster.
            chunks = [(0, 256), (256, 1024)]
            off = 1280
            while off < N_LOCAL:
                sz = min(CHUNK_PTS, N_LOCAL - off)
                chunks.append((off, sz))
                off += sz
            first = True
            for c0, csz in chunks:
                xat = xap.tile([N_ROWS, csz], bf16)
                if first:
                    nc.sync.dma_start(out=xat[:], in_=xa[:, c0 : c0 + csz])
                else:
                    nc.gpsimd.dma_start(out=xat[:], in_=xa[:, c0 : c0 + csz])
                last_chunk = c0 + csz == N_LOCAL
                n_iters = csz // ITER_PTS
                for it in range(n_iters):
                    p0 = it * ITER_PTS
                    g0 = c0 + p0
                    if (last_chunk and it == n_iters - 1) or (first and it == 0):
                        emit_iter(g0, xat, p0, 1, warm=first)
                        emit_iter(g0 + 128, xat, p0 + 128, 1)
                    else:
                        emit_iter(g0, xat, p0, 2)
                first = False
    return nc


def _split_bf16(a32):
    """fp32 array -> (hi, lo) bf16 pair with hi + lo ~= a32."""
    import ml_dtypes

    hi = a32.astype(ml_dtypes.bfloat16)
    lo = (a32 - hi.astype(np.float32)).astype(ml_dtypes.bfloat16)
    return hi, lo


def _prep_operands(normal, kern):
    """Build the rank-13 augmented bf16 operands so that
    (xa.T @ ka)[i, j] ~= 2 x_i.k_j - |x_i|^2 - |k_j|^2 = -d2[i, j]."""
    import ml_dtypes

    x = np.ascontiguousarray(np.asarray(normal, dtype=np.float32))  # (n, 3)
    kf = np.asarray(kern, dtype=np.float32).reshape(MK, 3)  # (1024, 3)

    n2 = (x * x).sum(axis=1)  # (n,)
    k2 = (kf * kf).sum(axis=1)  # (1024,)

    xhi, xlo = _split_bf16(x)
    khi, klo = _split_bf16(kf)
    n2hi, n2lo = _split_bf16(n2)
    k2hi, k2lo = _split_bf16(k2)

    n = x.shape[0]
    ones_n = np.ones(n, dtype=ml_dtypes.bfloat16)
    ones_k = np.ones(MK, dtype=ml_dtypes.bfloat16)
    two_khi = (2.0 * khi.astype(np.float32)).astype(ml_dtypes.bfloat16)  # exact
    two_klo = (2.0 * klo.astype(np.float32)).astype(ml_dtypes.bfloat16)  # exact

    # row r of xa pairs with row r of ka; sum over the 13 rows gives -d2;
    # the -ln(128) output scale rides the ACT bias, so the device exp
    # yields exp(-d2)/128.
    xa = np.empty((N_ROWS, n), dtype=ml_dtypes.bfloat16)
    ka = np.empty((N_ROWS, MK), dtype=ml_dtypes.bfloat16)
    xa[0:3] = xhi.T
    ka[0:3] = two_khi.T
    xa[3:6] = xhi.T
    ka[3:6] = two_klo.T
    xa[6:9] = xlo.T
    ka[6:9] = two_khi.T
    xa[9] = -n2hi
    ka[9] = ones_k
    xa[10] = -n2lo
    ka[10] = ones_k
    xa[11] = ones_n
    ka[11] = -k2hi
    xa[12] = ones_n
    ka[12] = -k2lo
    return xa, ka


def kernel(normal, neighbour, kernel):  # noqa: A002 - harness-fixed names
    global _CACHED_NC, LAST_RESULTS
    from concourse.bass_utils import run_bass_kernel_spmd

    xa, ka = _prep_operands(normal, kernel)
    assert xa.shape[1] == N_TOTAL, xa.shape

    if _CACHED_NC is None:
        _CACHED_NC = _build_bass()
        if not _CACHED_NC.is_finalized():
            _CACHED_NC.finalize()

    in_maps = [
        {
            "xa": np.ascontiguousarray(xa[:, i * N_LOCAL : (i + 1) * N_LOCAL]),
            "ka": ka,
        }
        for i in range(N_CORES)
    ]
    res = run_bass_kernel_spmd(
        _CACHED_NC, in_maps, list(range(N_CORES)), trace=TRACE
    )
    LAST_RESULTS = res
    out = np.concatenate(
        [res.results[i]["out"] for i in range(N_CORES)], axis=0
    )
    return np.ascontiguousarray(out.astype(np.float32))



# revision 47
# speedup vs baseline: 1.0179x; 1.0033x over previous
"""Kernel-correlation (gnn_message_passing) Trainium2 kernel.

out[i, m] = (1/128) * sum_{l<16} exp(-||normal[i] - kernel[m, l]||^2)

Strategy (data-parallel over points, 8 NeuronCores, no collectives):
  -d2[i, j] = 2 x_i . k_j - |x_i|^2 - |k_j|^2 is a rank-13 product of two
  bf16 hi/lo-split augmented matrices (fp32-grade accuracy at full bf16 PE
  rate), computed straight into PSUM by the TensorEngine.  ScalarE then does
  exp(psum - ln 128) PSUM->SBUF bf16 in one pass (the 1/128 output scale
  rides the per-partition activation bias), and VectorE does the grouped
  16->1 reduction as a 4-level pairwise bf16 add tree (packed 2-byte
  operands engage the DVE 2x perf mode; the plain tensor_reduce runs
  mode-less at 1x and was the original critical path).  Output rows DMA
  back contiguously.

Per core: 32768 points x 1024 kernel-points = 33.5M exp evals.  ACT is the
bound engine: 128 instrs x (2048+222)*0.833ns = 242.6us busy (~96% of the
kernel span); DVE tree ~168us, PE ~114us.  Pipeline head/tail trimmed via a
ramped chunk schedule, split kernel-tile DMAs on separate queues, an ACT
Exp-table warm-up, PE p-state pre-warm matmuls, and output DMAs alternating
between the SP and Pool queues so tail dispatches overlap.  Timeline-sim:
251.7us (was 300.1us with the mode-less tensor_reduce on the critical
path).
"""

import math

import numpy as np

N_TOTAL = 262144
N_CORES = 8
N_LOCAL = N_TOTAL // N_CORES  # 32768
M_KERN = 64
K_SUB = 16
MK = M_KERN * K_SUB  # 1024
N_ROWS = 13  # 9 hi/lo cross terms + n2 hi/lo + k2 hi/lo (ln128 rides ACT bias)
CHUNK_PTS = 2048  # points per input DMA chunk
ITER_PTS = 256  # points per PSUM iteration (2 tiles of 128)
NEG_LN128 = -float(math.log(128.0))  # exp output scale, applied as ACT bias

TRACE = False  # set by test.py to collect a neuron profile
LAST_RESULTS = None  # BassKernelResults of the most recent run

_CACHED_NC = None


def _build_bass():
    import concourse.bacc as bacc
    import concourse.mybir as mybir
    from concourse.tile import TileContext

    f32 = mybir.dt.float32
    bf16 = mybir.dt.bfloat16
    EXP = mybir.ActivationFunctionType.Exp

    nc = bacc.Bacc()
    xa = nc.declare_dram_parameter("xa", [N_ROWS, N_LOCAL], bf16, isOutput=False)
    ka = nc.declare_dram_parameter("ka", [N_ROWS, MK], bf16, isOutput=False)
    out = nc.declare_dram_parameter("out", [N_LOCAL, M_KERN], f32, isOutput=True)

    with TileContext(nc) as tc:
        with (
            tc.tile_pool(name="kap", bufs=1) as kap,
            tc.tile_pool(name="xap", bufs=3) as xap,
            tc.tile_pool(name="valsp", bufs=3) as valsp,
            tc.tile_pool(name="tre1p", bufs=3) as tre1p,
            tc.tile_pool(name="tre2p", bufs=3) as tre2p,
            tc.tile_pool(name="tre3p", bufs=3) as tre3p,
            tc.tile_pool(name="outp", bufs=4) as outp,
            tc.tile_pool(name="psump", bufs=2, space="PSUM") as psump,
        ):
            # Per-partition ACT bias tile holding -ln(128); doubles as the
            # warm-up input so the Exp table loads during the initial DMAs.
            biast = kap.tile([128, 1], f32)
            warm2 = kap.tile([128, 1], f32)
            nc.vector.memset(biast[:], NEG_LN128)
            nc.scalar.activation(out=warm2[:], in_=biast[:], func=EXP)

            # Pre-warm the PE p-state with dummy matmuls on scratch data so
            # the first real iterations run at full clock (the cold PE runs
            # ~2-4x slower and was stalling the ACT stream by ~2us).
            scr_l = kap.tile([N_ROWS, 128], bf16)
            scr_r = kap.tile([N_ROWS, 512], bf16)
            nc.vector.memset(scr_l[:], 0.0)
            nc.vector.memset(scr_r[:], 0.0)
            # Two half-width kernel tiles so the first matmul only waits for
            # a 15KB DMA, not the full 30KB.
            kat0 = kap.tile([N_ROWS, 512], bf16)
            kat1 = kap.tile([N_ROWS, 512], bf16)
            kats = [kat0, kat1]
            # First DMAs go out on separate queues so their issue overlaps;
            # kat1 is issued after the first point chunk (it isn't needed
            # until the second matmul of the first iteration).
            nc.gpsimd.dma_start(out=kats[0][:], in_=ka[:, 0:512])

            def emit_iter(g0, xat, p0, nhalves, warm=False):
                w = nhalves * MK
                ps = psump.tile([128, 2 * MK], f32)
                if warm:
                    # The first (128-pt) iteration only uses ps[:, 0:MK];
                    # run dummy matmuls into the unused half while the first
                    # DMAs are in flight so the PE p-state ramps up before
                    # real data arrives.
                    for _ in range(4):
                        nc.tensor.matmul(
                            out=ps[:, MK : MK + 512],
                            lhsT=scr_l[:],
                            rhs=scr_r[:],
                            start=True,
                            stop=True,
                        )
                for half in range(nhalves):
                    lhsT = xat[:, p0 + half * 128 : p0 + (half + 1) * 128]
                    for jb in range(2):
                        nc.tensor.matmul(
                            out=ps[
                                :, half * MK + jb * 512 : half * MK + (jb + 1) * 512
                            ],
                            lhsT=lhsT,
                            rhs=kats[jb][:],
                            start=True,
                            stop=True,
                        )
                vals = valsp.tile([128, w], bf16)
                nc.scalar.activation(
                    out=vals[:], in_=ps[:, 0:w], func=EXP, bias=biast[:]
                )
                # 16 -> 1 grouped sum as a pairwise add tree.  All-bf16
                # packed operands hit the DVE 2x perf mode (the plain
                # tensor_reduce runs mode-less at 1x and was the
                # kernel-wide critical path at 2.2us/tile).
                t1 = tre1p.tile([128, w // 2], bf16)
                t2 = tre2p.tile([128, w // 4], bf16)
                t3 = tre3p.tile([128, w // 8], bf16)
                ot = outp.tile([128, w // 16], f32)
                v3 = vals[:].rearrange("p (g l) -> p g l", l=16)
                a3 = t1[:].rearrange("p (g l) -> p g l", l=8)
                b3 = t2[:].rearrange("p (g l) -> p g l", l=4)
                c3 = t3[:].rearrange("p (g l) -> p g l", l=2)
                with nc.allow_low_precision(reason="bf16 16->1 sum, tol 2e-2"):
                    nc.vector.tensor_add(a3[:], v3[:, :, 0:8], v3[:, :, 8:16])
                    nc.vector.tensor_add(b3[:], a3[:, :, 0:4], a3[:, :, 4:8])
                    nc.vector.tensor_add(c3[:], b3[:, :, 0:2], b3[:, :, 2:4])
                    nc.vector.tensor_add(
                        ot[:].rearrange("p (g l) -> p g l", l=1),
                        c3[:, :, 0:1],
                        c3[:, :, 1:2],
                    )
                for half in range(nhalves):
                    # Alternate output DMAs between the SP and Pool queues so
                    # dispatches overlap (matters at the pipeline tail where
                    # the final DMAs otherwise serialize on one sequencer).
                    q = nc.sync if (g0 // 128 + half) % 2 == 1 else nc.gpsimd
                    q.dma_start(
                        out=out[g0 + half * 128 : g0 + (half + 1) * 128, :],
                        in_=ot[:, half * M_KERN : (half + 1) * M_KERN],
                    )

            # Ramp the chunk schedule (256, 1024, then 2048s) so compute
            # starts ~3us sooner and the second chunk lands before the first
            # runs dry; end with two 128-pt iterations so the tail drains
            # faster.
            chunks = [(0, 256), (256, 1024)]
            off = 1280
            while off < N_LOCAL:
                sz = min(CHUNK_PTS, N_LOCAL - off)
                chunks.append((off, sz))
                off += sz
            first = True
            for c0, csz in chunks:
                xat = xap.tile([N_ROWS, csz], bf16)
                if first:
                    nc.sync.dma_start(out=xat[:], in_=xa[:, c0 : c0 + csz])
                    nc.sync.dma_start(out=kats[1][:], in_=ka[:, 512:1024])
                else:
                    nc.gpsimd.dma_start(out=xat[:], in_=xa[:, c0 : c0 + csz])
                last_chunk = c0 + csz == N_LOCAL
                n_iters = csz // ITER_PTS
                for it in range(n_iters):
                    p0 = it * ITER_PTS
                    g0 = c0 + p0
                    if (last_chunk and it == n_iters - 1) or (first and it == 0):
                        emit_iter(g0, xat, p0, 1, warm=first)
                        emit_iter(g0 + 128, xat, p0 + 128, 1)
                    else:
                        emit_iter(g0, xat, p0, 2)
                first = False
    return nc


def _split_bf16(a32):
    """fp32 array -> (hi, lo) bf16 pair with hi + lo ~= a32."""
    import ml_dtypes

    hi = a32.astype(ml_dtypes.bfloat16)
    lo = (a32 - hi.astype(np.float32)).astype(ml_dtypes.bfloat16)
    return hi, lo


def _prep_operands(normal, kern):
    """Build the rank-13 augmented bf16 operands so that
    (xa.T @ ka)[i, j] ~= 2 x_i.k_j - |x_i|^2 - |k_j|^2 = -d2[i, j]."""
    import ml_dtypes

    x = np.ascontiguousarray(np.asarray(normal, dtype=np.float32))  # (n, 3)
    kf = np.asarray(kern, dtype=np.float32).reshape(MK, 3)  # (1024, 3)

    n2 = (x * x).sum(axis=1)  # (n,)
    k2 = (kf * kf).sum(axis=1)  # (1024,)

    xhi, xlo = _split_bf16(x)
    khi, klo = _split_bf16(kf)
    n2hi, n2lo = _split_bf16(n2)
    k2hi, k2lo = _split_bf16(k2)

    n = x.shape[0]
    ones_n = np.ones(n, dtype=ml_dtypes.bfloat16)
    ones_k = np.ones(MK, dtype=ml_dtypes.bfloat16)
    two_khi = (2.0 * khi.astype(np.float32)).astype(ml_dtypes.bfloat16)  # exact
    two_klo = (2.0 * klo.astype(np.float32)).astype(ml_dtypes.bfloat16)  # exact

    # row r of xa pairs with row r of ka; sum over the 13 rows gives -d2;
    # the -ln(128) output scale rides the ACT bias, so the device exp
    # yields exp(-d2)/128.
    xa = np.empty((N_ROWS, n), dtype=ml_dtypes.bfloat16)
    ka = np.empty((N_ROWS, MK), dtype=ml_dtypes.bfloat16)
    xa[0:3] = xhi.T
    ka[0:3] = two_khi.T
    xa[3:6] = xhi.T
    ka[3:6] = two_klo.T
    xa[6:9] = xlo.T
    ka[6:9] = two_khi.T
    xa[9] = -n2hi
    ka[9] = ones_k
    xa[10] = -n2lo
    ka[10] = ones_k
    xa[11] = ones_n
    ka[11] = -k2hi
    xa[12] = ones_n
    ka[12] = -k2lo
    return xa, ka


def kernel(normal, neighbour, kernel):  # noqa: A002 - harness-fixed names
    global _CACHED_NC, LAST_RESULTS
    from concourse.bass_utils import run_bass_kernel_spmd

    xa, ka = _prep_operands(normal, kernel)
    assert xa.shape[1] == N_TOTAL, xa.shape

    if _CACHED_NC is None:
        _CACHED_NC = _build_bass()
        if not _CACHED_NC.is_finalized():
            _CACHED_NC.finalize()

    in_maps = [
        {
            "xa": np.ascontiguousarray(xa[:, i * N_LOCAL : (i + 1) * N_LOCAL]),
            "ka": ka,
        }
        for i in range(N_CORES)
    ]
    res = run_bass_kernel_spmd(
        _CACHED_NC, in_maps, list(range(N_CORES)), trace=TRACE
    )
    LAST_RESULTS = res
    out = np.concatenate(
        [res.results[i]["out"] for i in range(N_CORES)], axis=0
    )
    return np.ascontiguousarray(out.astype(np.float32))



# revision 52
# speedup vs baseline: 1.0180x; 1.0001x over previous
"""Kernel-correlation (gnn_message_passing) Trainium2 kernel.

out[i, m] = (1/128) * sum_{l<16} exp(-||normal[i] - kernel[m, l]||^2)

Strategy (data-parallel over points, 8 NeuronCores, no collectives):
  -d2[i, j] = 2 x_i . k_j - |x_i|^2 - |k_j|^2 is a rank-13 product of two
  bf16 hi/lo-split augmented matrices (fp32-grade accuracy at full bf16 PE
  rate), computed straight into PSUM by the TensorEngine.  ScalarE then does
  exp(psum - ln 128) PSUM->SBUF bf16 in one pass (the 1/128 output scale
  rides the per-partition activation bias), and VectorE does the grouped
  16->1 reduction as a 4-level pairwise bf16 add tree (packed 2-byte
  operands engage the DVE 2x perf mode; the plain tensor_reduce runs
  mode-less at 1x and was the original critical path).  Output rows DMA
  back contiguously.

Per core: 32768 points x 1024 kernel-points = 33.5M exp evals.  ACT is the
bound engine: 128 instrs x (2048+222)*0.833ns = 242.6us busy (~96% of the
kernel span); DVE tree ~168us, PE ~114us.  Pipeline head/tail trimmed via a
ramped chunk schedule, split kernel-tile DMAs on separate queues, an ACT
Exp-table warm-up, PE p-state pre-warm matmuls, and output DMAs alternating
between the SP and Pool queues so tail dispatches overlap.  Timeline-sim:
251.7us (was 300.1us with the mode-less tensor_reduce on the critical
path).
"""

import math

import numpy as np

N_TOTAL = 262144
N_CORES = 8
N_LOCAL = N_TOTAL // N_CORES  # 32768
M_KERN = 64
K_SUB = 16
MK = M_KERN * K_SUB  # 1024
N_ROWS = 13  # 9 hi/lo cross terms + n2 hi/lo + k2 hi/lo (ln128 rides ACT bias)
CHUNK_PTS = 2048  # points per input DMA chunk
ITER_PTS = 256  # points per PSUM iteration (2 tiles of 128)
NEG_LN128 = -float(math.log(128.0))  # exp output scale, applied as ACT bias

TRACE = False  # set by test.py to collect a neuron profile
LAST_RESULTS = None  # BassKernelResults of the most recent run

_CACHED_NC = None


def _build_bass():
    import concourse.bacc as bacc
    import concourse.mybir as mybir
    from concourse.tile import TileContext

    f32 = mybir.dt.float32
    bf16 = mybir.dt.bfloat16
    EXP = mybir.ActivationFunctionType.Exp

    nc = bacc.Bacc()
    xa = nc.declare_dram_parameter("xa", [N_ROWS, N_LOCAL], bf16, isOutput=False)
    ka = nc.declare_dram_parameter("ka", [N_ROWS, MK], bf16, isOutput=False)
    out = nc.declare_dram_parameter("out", [N_LOCAL, M_KERN], f32, isOutput=True)

    with TileContext(nc) as tc:
        with (
            tc.tile_pool(name="kap", bufs=1) as kap,
            tc.tile_pool(name="xap", bufs=3) as xap,
            tc.tile_pool(name="valsp", bufs=3) as valsp,
            tc.tile_pool(name="tre1p", bufs=3) as tre1p,
            tc.tile_pool(name="tre2p", bufs=3) as tre2p,
            tc.tile_pool(name="tre3p", bufs=3) as tre3p,
            tc.tile_pool(name="outp", bufs=4) as outp,
            tc.tile_pool(name="psump", bufs=2, space="PSUM") as psump,
        ):
            # Per-partition ACT bias tile holding -ln(128); doubles as the
            # warm-up input so the Exp table loads during the initial DMAs.
            biast = kap.tile([128, 1], f32)
            warm2 = kap.tile([128, 1], f32)
            nc.vector.memset(biast[:], NEG_LN128)
            nc.scalar.activation(out=warm2[:], in_=biast[:], func=EXP)

            # Pre-warm the PE p-state with dummy matmuls on scratch data so
            # the first real iterations run at full clock (the cold PE runs
            # ~2-4x slower and was stalling the ACT stream by ~2us).
            scr_l = kap.tile([N_ROWS, 128], bf16)
            scr_r = kap.tile([N_ROWS, 512], bf16)
            nc.vector.memset(scr_l[:], 0.0)
            nc.vector.memset(scr_r[:], 0.0)
            # Two half-width kernel tiles so the first matmul only waits for
            # a 15KB DMA, not the full 30KB.
            kat0 = kap.tile([N_ROWS, 512], bf16)
            kat1 = kap.tile([N_ROWS, 512], bf16)
            kats = [kat0, kat1]
            # First DMAs go out on separate queues so their issue overlaps;
            # kat1 is issued after the first point chunk (it isn't needed
            # until the second matmul of the first iteration).
            nc.gpsimd.dma_start(out=kats[0][:], in_=ka[:, 0:512])

            def emit_iter(g0, xat, p0, nhalves, warm=False):
                w = nhalves * MK
                ps = psump.tile([128, 2 * MK], f32)
                if warm:
                    # The first (128-pt) iteration only uses ps[:, 0:MK];
                    # run dummy matmuls into the unused half while the first
                    # DMAs are in flight so the PE p-state ramps up before
                    # real data arrives.
                    for _ in range(4):
                        nc.tensor.matmul(
                            out=ps[:, MK : MK + 512],
                            lhsT=scr_l[:],
                            rhs=scr_r[:],
                            start=True,
                            stop=True,
                        )
                for half in range(nhalves):
                    lhsT = xat[:, p0 + half * 128 : p0 + (half + 1) * 128]
                    for jb in range(2):
                        nc.tensor.matmul(
                            out=ps[
                                :, half * MK + jb * 512 : half * MK + (jb + 1) * 512
                            ],
                            lhsT=lhsT,
                            rhs=kats[jb][:],
                            start=True,
                            stop=True,
                        )
                vals = valsp.tile([128, w], bf16)
                nc.scalar.activation(
                    out=vals[:], in_=ps[:, 0:w], func=EXP, bias=biast[:]
                )
                # 16 -> 1 grouped sum as a pairwise add tree.  All-bf16
                # packed operands hit the DVE 2x perf mode (the plain
                # tensor_reduce runs mode-less at 1x and was the
                # kernel-wide critical path at 2.2us/tile).
                t1 = tre1p.tile([128, w // 2], bf16)
                t2 = tre2p.tile([128, w // 4], bf16)
                t3 = tre3p.tile([128, w // 8], bf16)
                ot = outp.tile([128, w // 16], f32)
                v3 = vals[:].rearrange("p (g l) -> p g l", l=16)
                a3 = t1[:].rearrange("p (g l) -> p g l", l=8)
                b3 = t2[:].rearrange("p (g l) -> p g l", l=4)
                c3 = t3[:].rearrange("p (g l) -> p g l", l=2)
                with nc.allow_low_precision(reason="bf16 16->1 sum, tol 2e-2"):
                    nc.vector.tensor_add(a3[:], v3[:, :, 0:8], v3[:, :, 8:16])
                    nc.vector.tensor_add(b3[:], a3[:, :, 0:4], a3[:, :, 4:8])
                    nc.vector.tensor_add(c3[:], b3[:, :, 0:2], b3[:, :, 2:4])
                    nc.vector.tensor_add(
                        ot[:].rearrange("p (g l) -> p g l", l=1),
                        c3[:, :, 0:1],
                        c3[:, :, 1:2],
                    )
                for half in range(nhalves):
                    # Alternate output DMAs between the SP and Pool queues so
                    # dispatches overlap (matters at the pipeline tail where
                    # the final DMAs otherwise serialize on one sequencer).
                    q = nc.sync if (g0 // 128 + half) % 2 == 1 else nc.gpsimd
                    q.dma_start(
                        out=out[g0 + half * 128 : g0 + (half + 1) * 128, :],
                        in_=ot[:, half * M_KERN : (half + 1) * M_KERN],
                    )

            def emit_half(g0, xat, p0, jb, warm=0):
                # 128 points x 512 kernel-cols: filters m in [32*jb, 32*jb+32).
                # Used only for the very first iteration so the ACT stream
                # starts after ONE matmul needing only kat0.
                ps = psump.tile([128, 2 * MK], f32)
                if warm:
                    for _ in range(warm):
                        nc.tensor.matmul(
                            out=ps[:, MK : MK + 512],
                            lhsT=scr_l[:],
                            rhs=scr_r[:],
                            start=True,
                            stop=True,
                        )
                nc.tensor.matmul(
                    out=ps[:, 0:512],
                    lhsT=xat[:, p0 : p0 + 128],
                    rhs=kats[jb][:],
                    start=True,
                    stop=True,
                )
                vals = valsp.tile([128, 512], bf16)
                nc.scalar.activation(
                    out=vals[:], in_=ps[:, 0:512], func=EXP, bias=biast[:]
                )
                t1 = tre1p.tile([128, 256], bf16)
                t2 = tre2p.tile([128, 128], bf16)
                t3 = tre3p.tile([128, 64], bf16)
                ot = outp.tile([128, 32], f32)
                v3 = vals[:].rearrange("p (g l) -> p g l", l=16)
                a3 = t1[:].rearrange("p (g l) -> p g l", l=8)
                b3 = t2[:].rearrange("p (g l) -> p g l", l=4)
                c3 = t3[:].rearrange("p (g l) -> p g l", l=2)
                with nc.allow_low_precision(reason="bf16 16->1 sum, tol 2e-2"):
                    nc.vector.tensor_add(a3[:], v3[:, :, 0:8], v3[:, :, 8:16])
                    nc.vector.tensor_add(b3[:], a3[:, :, 0:4], a3[:, :, 4:8])
                    nc.vector.tensor_add(c3[:], b3[:, :, 0:2], b3[:, :, 2:4])
                    nc.vector.tensor_add(
                        ot[:].rearrange("p (g l) -> p g l", l=1),
                        c3[:, :, 0:1],
                        c3[:, :, 1:2],
                    )
                q = nc.sync if jb == 1 else nc.gpsimd
                q.dma_start(
                    out=out[g0 : g0 + 128, 32 * jb : 32 * jb + 32],
                    in_=ot[:],
                )

            # Ramp the chunk schedule (256, 1024, then 2048s) so compute
            # starts ~3us sooner and the second chunk lands before the first
            # runs dry; end with two 128-pt iterations so the tail drains
            # faster.
            chunks = [(0, 256), (256, 1024)]
            off = 1280
            while off < N_LOCAL:
                sz = min(CHUNK_PTS, N_LOCAL - off)
                chunks.append((off, sz))
                off += sz
            first = True
            for c0, csz in chunks:
                xat = xap.tile([N_ROWS, csz], bf16)
                if first:
                    nc.sync.dma_start(out=xat[:], in_=xa[:, c0 : c0 + csz])
                    nc.sync.dma_start(out=kats[1][:], in_=ka[:, 512:1024])
                else:
                    nc.gpsimd.dma_start(out=xat[:], in_=xa[:, c0 : c0 + csz])
                last_chunk = c0 + csz == N_LOCAL
                n_iters = csz // ITER_PTS
                for it in range(n_iters):
                    p0 = it * ITER_PTS
                    g0 = c0 + p0
                    if first and it == 0:
                        emit_half(g0, xat, p0, 0, warm=2)
                        emit_half(g0, xat, p0, 1, warm=2)
                        emit_iter(g0 + 128, xat, p0 + 128, 1)
                    elif last_chunk and it == n_iters - 1:
                        emit_iter(g0, xat, p0, 1)
                        emit_iter(g0 + 128, xat, p0 + 128, 1)
                    else:
                        emit_iter(g0, xat, p0, 2)
                first = False
    return nc


def _split_bf16(a32):
    """fp32 array -> (hi, lo) bf16 pair with hi + lo ~= a32."""
    import ml_dtypes

    hi = a32.astype(ml_dtypes.bfloat16)
    lo = (a32 - hi.astype(np.float32)).astype(ml_dtypes.bfloat16)
    return hi, lo


def _prep_operands(normal, kern):
    """Build the rank-13 augmented bf16 operands so that
    (xa.T @ ka)[i, j] ~= 2 x_i.k_j - |x_i|^2 - |k_j|^2 = -d2[i, j]."""
    import ml_dtypes

    x = np.ascontiguousarray(np.asarray(normal, dtype=np.float32))  # (n, 3)
    kf = np.asarray(kern, dtype=np.float32).reshape(MK, 3)  # (1024, 3)

    n2 = (x * x).sum(axis=1)  # (n,)
    k2 = (kf * kf).sum(axis=1)  # (1024,)

    xhi, xlo = _split_bf16(x)
    khi, klo = _split_bf16(kf)
    n2hi, n2lo = _split_bf16(n2)
    k2hi, k2lo = _split_bf16(k2)

    n = x.shape[0]
    ones_n = np.ones(n, dtype=ml_dtypes.bfloat16)
    ones_k = np.ones(MK, dtype=ml_dtypes.bfloat16)
    two_khi = (2.0 * khi.astype(np.float32)).astype(ml_dtypes.bfloat16)  # exact
    two_klo = (2.0 * klo.astype(np.float32)).astype(ml_dtypes.bfloat16)  # exact

    # row r of xa pairs with row r of ka; sum over the 13 rows gives -d2;
    # the -ln(128) output scale rides the ACT bias, so the device exp
    # yields exp(-d2)/128.
    xa = np.empty((N_ROWS, n), dtype=ml_dtypes.bfloat16)
    ka = np.empty((N_ROWS, MK), dtype=ml_dtypes.bfloat16)
    xa[0:3] = xhi.T
    ka[0:3] = two_khi.T
    xa[3:6] = xhi.T
    ka[3:6] = two_klo.T
    xa[6:9] = xlo.T
    ka[6:9] = two_khi.T
    xa[9] = -n2hi
    ka[9] = ones_k
    xa[10] = -n2lo
    ka[10] = ones_k
    xa[11] = ones_n
    ka[11] = -k2hi
    xa[12] = ones_n
    ka[12] = -k2lo
    return xa, ka


def kernel(normal, neighbour, kernel):  # noqa: A002 - harness-fixed names
    global _CACHED_NC, LAST_RESULTS
    from concourse.bass_utils import run_bass_kernel_spmd

    xa, ka = _prep_operands(normal, kernel)
    assert xa.shape[1] == N_TOTAL, xa.shape

    if _CACHED_NC is None:
        _CACHED_NC = _build_bass()
        if not _CACHED_NC.is_finalized():
            _CACHED_NC.finalize()

    in_maps = [
        {
            "xa": np.ascontiguousarray(xa[:, i * N_LOCAL : (i + 1) * N_LOCAL]),
            "ka": ka,
        }
        for i in range(N_CORES)
    ]
    res = run_bass_kernel_spmd(
        _CACHED_NC, in_maps, list(range(N_CORES)), trace=TRACE
    )
    LAST_RESULTS = res
    out = np.concatenate(
        [res.results[i]["out"] for i in range(N_CORES)], axis=0
    )
    return np.ascontiguousarray(out.astype(np.float32))

